# revision 1
# baseline (speedup 1.0000x reference)
"""Bass/Trainium2 kernel for nn_Graph_Layer (gnn_message_passing).

Reference math (N=8192, D=512):
    G0[i,j] = ||s_i - s_j + eps||_2   (pairwise distances, Gram trick)
    G = 1 - G0 / rowmax(G0)
    out = (G @ x) @ W

Decomposition used here (row-shard over 8 cores, 1024 rows each):
    sqd[i,j] = ri[i] + cj[j] - 2*gram[i,j]        (ri, cj host-precomputed)
    G0 = sqrt(sqd + CLAMP)                         (CLAMP covers tf32 noise on diag)
    rowmax[i] = max_j G0[i,j]
    (G @ x)[i,:] = colsum_x - Y0[i,:]/rowmax[i],   Y0 = G0 @ x
    out[i,:]  = w2 - (Y0[i,:]/rowmax[i]) @ W,      w2 = colsum_x @ W (host)

On device, the distance strip is computed TRANSPOSED (sqd^T[j,i]) so that the
G0 tiles come out with j (the contraction dim of Y0 = G0 @ x) on partitions --
no transposes of G0 needed. Each core sees its own np.roll'ed copy of the
inputs so the "local rows" are always rows [0,1024): a single uniform SPMD
program runs on all 8 cores.

All matmuls use float32r (TF32 mode: 1 cycle/row at free dim >= 512).
"""

import numpy as np
from contextlib import ExitStack

import concourse.bass as bass
from concourse import bacc
import concourse.tile as tile
from concourse import mybir
from concourse.bass_utils import run_bass_kernel_spmd
from concourse.masks import make_identity

N, D, NOUT = 8192, 512, 512
M = 8                 # cores
R = N // M            # 1024 local rows per core
EPS = 1e-6
CLAMP = 0.3           # covers tf32 rounding noise on the diagonal; ~1e-4 rel effect off-diag
F32 = mybir.dt.float32
F32R = mybir.dt.float32r

KT = D // 128         # 4 contraction sub-tiles
NJT = N // 128        # 64 j tiles
IB = 512              # i block (free dim of the gram matmuls)
NIB = R // IB         # 2
NSUB = IB // 128      # 4 sub-tiles of 128 rows per i block

CH = 512              # S^T DMA chunk width (columns); chunk c covers j_tiles 4c..4c+3
NCH = N // CH
LOOKAHEAD = 4         # chunks issued ahead of consumption


def build_kernel(ctx, tc, out_d, x_d, s_d, cj_d, ri_d, w_d):
    nc = tc.nc

    singles = ctx.enter_context(tc.tile_pool(name="singles", bufs=1))
    xt_pool = ctx.enter_context(tc.tile_pool(name="xt", bufs=4))
    g0_pool = ctx.enter_context(tc.tile_pool(name="g0", bufs=3))
    ysc_pool = ctx.enter_context(tc.tile_pool(name="ysc", bufs=4))
    yscT_pool = ctx.enter_context(tc.tile_pool(name="ysct", bufs=2))
    osb_pool = ctx.enter_context(tc.tile_pool(name="osb", bufs=2))
    sm_pool = ctx.enter_context(tc.tile_pool(name="sm", bufs=4))
    macc_pool = ctx.enter_context(tc.tile_pool(name="macc", bufs=2))
    ps_tr = ctx.enter_context(tc.tile_pool(name="ps_tr", bufs=2, space="PSUM"))
    ps_g = ctx.enter_context(tc.tile_pool(name="ps_g", bufs=2, space="PSUM"))
    ps_y = ctx.enter_context(tc.tile_pool(name="ps_y", bufs=1, space="PSUM"))

    # --- persistent SBUF tensors ---
    st = singles.tile([128, KT * N], F32R)            # S^T: [k*N + j] layout
    w_sb = singles.tile([128, 5 * NOUT], F32R)        # W rows 0..511 + w2 row (tile 4, part 0)
    cj_sb = singles.tile([128, NJT], F32)             # cj[t*128+p] at [p, t]
    ri_sb = singles.tile([1, R], F32R)                # -ri/2
    ones_sb = singles.tile([1, 128], F32R)
    ident = singles.tile([128, 128], F32)

    ones32 = singles.tile([1, 128], F32)
    nc.vector.memset(ones32[:], 1.0)
    nc.vector.tensor_copy(out=ones_sb[:], in_=ones32[:])
    make_identity(nc, ident[:])
    def load_st_chunk(c):
        for k in range(KT):
            nc.sync.dma_start(
                out=st[:, k * N + c * CH: k * N + (c + 1) * CH],
                in_=s_d[bass.ts(k, 128), c * CH:(c + 1) * CH].bitcast(F32R),
            )

    nc.sync.dma_start(out=ri_sb[:], in_=ri_d.bitcast(F32R))
    load_st_chunk(0)
    nc.sync.dma_start(out=cj_sb[:], in_=cj_d)

    # --- main: per i-block: gram strip -> G0 -> Y0 accum -> normalize -> GEMM ---
    for ib in range(NIB):
        icol0 = ib * IB  # local column offset into S^T / ri
        psy = [ps_y.tile([128, NOUT], F32, tag=f"y{s}", name=f"psy{s}")
               for s in range(NSUB)]
        macc = macc_pool.tile([128, IB], F32, tag="macc")

        for jt in range(NJT):
            xt = xt_pool.tile([128, D], F32R, tag="xt")
            nc.sync.dma_start(out=xt[:], in_=x_d[bass.ts(jt, 128), :].bitcast(F32R))

            if ib == 0:
                if jt == 0:
                    load_st_chunk(1)
                    load_st_chunk(2)
                elif jt % 4 == 0 and jt // 4 + 2 < NCH:
                    load_st_chunk(jt // 4 + 2)
                if jt == 32:
                    for kt in range(5):
                        nc.sync.dma_start(
                            out=w_sb[:, kt * NOUT:(kt + 1) * NOUT],
                            in_=w_d[bass.ts(kt, 128), :].bitcast(F32R),
                        )

            psg = ps_g.tile([128, IB], F32, tag="g")
            for k in range(KT):
                nc.tensor.matmul(
                    psg[:],
                    st[:, k * N + jt * 128: k * N + jt * 128 + 128],
                    st[:, k * N + icol0: k * N + icol0 + IB],
                    start=(k == 0),
                    stop=False,
                )
            # aug row: += 1 * (-ri[i]/2)
            nc.tensor.matmul(
                psg[:], ones_sb[:], ri_sb[:, icol0:icol0 + IB],
                start=False, stop=True,
            )

            # G0^T tile = sqrt(-2*psg + cj[j])   (cj includes +CLAMP)
            g0 = g0_pool.tile([128, IB], F32R, tag="g0")
            nc.scalar.activation(
                out=g0[:], in_=psg[:],
                func=mybir.ActivationFunctionType.Sqrt,
                bias=cj_sb[:, jt:jt + 1], scale=-2.0,
            )

            if jt == 0:
                nc.vector.tensor_copy(out=macc[:], in_=g0[:].bitcast(F32))
            else:
                nc.vector.tensor_max(macc[:], macc[:], g0[:].bitcast(F32))

            # software pipeline: issue Y matmuls one step behind the gram so
            # the PE fills the ACT sqrt latency with the next gram
            if jt > 0:
                pg0, pxt = prev
                for s in range(NSUB):
                    nc.tensor.matmul(
                        psy[s][:], pg0[:, bass.ts(s, 128)], pxt[:],
                        start=(jt == 1), stop=False,
                    )
            prev = (g0, xt)

        pg0, pxt = prev
        for s in range(NSUB):
            nc.tensor.matmul(
                psy[s][:], pg0[:, bass.ts(s, 128)], pxt[:],
                start=False, stop=True,
            )

        # tail, part 1: rowmax -> -1/rowmax -> scale Y out of PSUM (frees psy fast)
        yscs = []
        for s in range(NSUB):
            pst = ps_tr.tile([128, 128], F32, tag="tr")
            nc.tensor.transpose(pst[:], macc[:, bass.ts(s, 128)], ident[:])
            rm = sm_pool.tile([128, 1], F32, tag="rm")
            nc.vector.tensor_reduce(
                out=rm[:], in_=pst[:], axis=mybir.AxisListType.X,
                op=mybir.AluOpType.max,
            )
            nrm = sm_pool.tile([128, 1], F32, tag="nrm")
            nc.vector.tensor_scalar_mul(nrm[:], rm[:], -1.0)
            ninv = sm_pool.tile([128, 1], F32, tag="ninv")
            nc.vector.reciprocal(ninv[:], nrm[:])  # -1/rowmax

            ysc = ysc_pool.tile([128, NOUT], F32, tag="ysc", name=f"ysc{s}")
            nc.scalar.activation(
                out=ysc[:], in_=psy[s][:],
                func=mybir.ActivationFunctionType.Copy, scale=ninv[:],
            )
            yscs.append(ysc)

        # tail, part 2: transpose Ysc and multiply by W (+ w2 aug row)
        for s in range(NSUB):
            ysc = yscs[s]
            ysct = yscT_pool.tile([128, KT * 128], F32R, tag="ysct")
            for k in range(KT):
                pst2 = ps_tr.tile([128, 128], F32, tag="tr")
                nc.tensor.transpose(pst2[:], ysc[:, bass.ts(k, 128)], ident[:])
                if k % 2 == 0:
                    nc.vector.tensor_copy(out=ysct[:, bass.ts(k, 128)], in_=pst2[:])
                else:
                    nc.scalar.copy(out=ysct[:, bass.ts(k, 128)], in_=pst2[:])

            pso = ps_g.tile([128, NOUT], F32, tag="g", name=f"pso{s}")
            for k in range(KT):
                nc.tensor.matmul(
                    pso[:],
                    ysct[:, bass.ts(k, 128)],
                    w_sb[:, k * NOUT:(k + 1) * NOUT],
                    start=(k == 0),
                    stop=False,
                )
            nc.tensor.matmul(
                pso[:], ones_sb[:], w_sb[0:1, 4 * NOUT:5 * NOUT],
                start=False, stop=True,
            )
            osb = osb_pool.tile([128, NOUT], F32, tag="osb")
            nc.vector.tensor_copy(out=osb[:], in_=pso[:])
            nc.sync.dma_start(out=out_d[bass.ts(ib * NSUB + s, 128), :], in_=osb[:])


_NC_CACHE = {}


def _build_nc():
    if "nc" in _NC_CACHE:
        return _NC_CACHE["nc"]
    nc = bacc.Bacc("TRN2", target_bir_lowering=False, debug=False, num_devices=M)
    x_d = nc.dram_tensor("x", [N, D], F32, kind="ExternalInput").ap()
    s_d = nc.dram_tensor("simT", [D, N], F32, kind="ExternalInput").ap()
    cj_d = nc.dram_tensor("cj", [128, NJT], F32, kind="ExternalInput").ap()
    ri_d = nc.dram_tensor("riaug", [1, R], F32, kind="ExternalInput").ap()
    w_d = nc.dram_tensor("waug", [640, NOUT], F32, kind="ExternalInput").ap()
    out_d = nc.dram_tensor("out", [R, NOUT], F32, kind="ExternalOutput").ap()
    with tile.TileContext(nc) as tc, ExitStack() as ctx:
        build_kernel(ctx, tc, out_d, x_d, s_d, cj_d, ri_d, w_d)
    nc.compile()
    _NC_CACHE["nc"] = nc
    return nc


def make_in_maps(x, sim_feat, weight):
    x = np.ascontiguousarray(x, dtype=np.float32)
    sim = np.ascontiguousarray(sim_feat, dtype=np.float32)
    w = np.ascontiguousarray(weight, dtype=np.float32)

    sim64 = sim.astype(np.float64)
    sq = (sim64 * sim64).sum(1)
    ss = sim64.sum(1)
    cj_full = (sq - 2.0 * EPS * ss + CLAMP).astype(np.float32)         # [N]
    ri_full = sq + 2.0 * EPS * ss + D * EPS * EPS                      # [N] f64
    colsum = x.astype(np.float64).sum(0)
    w2 = (colsum @ w.astype(np.float64)).astype(np.float32)
    waug = np.zeros((640, NOUT), np.float32)
    waug[:D] = w
    waug[D] = w2

    in_maps = []
    for c in range(M):
        shift = c * R
        sim_c = np.ascontiguousarray(np.roll(sim, -shift, axis=0).T)
        x_c = np.roll(x, -shift, axis=0)
        cj_c = np.ascontiguousarray(
            np.roll(cj_full, -shift).reshape(NJT, 128).T
        )                                                               # [128, NJT]
        ri_c = np.ascontiguousarray(
            (-(ri_full[shift:shift + R]) / 2.0).astype(np.float32).reshape(1, R)
        )
        in_maps.append(
            {"x": x_c, "simT": sim_c, "cj": cj_c, "riaug": ri_c, "waug": waug}
        )
    return in_maps


def kernel(x, sim_feat, weight, _trace=False, **kw):
    nc = _build_nc()
    in_maps = make_in_maps(x, sim_feat, weight)
    res = run_bass_kernel_spmd(nc, in_maps, list(range(M)), trace=_trace, **kw)
    out = np.concatenate([res.results[c]["out"] for c in range(M)], axis=0)
    if _trace:
        return out, res
    return out



# revision 2
# speedup vs baseline: 1.4523x; 1.4523x over previous
"""Bass/Trainium2 kernel for nn_Graph_Layer (gnn_message_passing).

Reference math (N=8192, D=512):
    G0[i,j] = ||s_i - s_j + eps||_2   (pairwise distances, Gram trick)
    G = 1 - G0 / rowmax(G0)
    out = (G @ x) @ W

Decomposition (row-shard over 8 cores, 1024 rows each). Key identity:
(G @ x) @ W = G @ (x @ W), so the weight GEMM folds into a host-side
precompute xw = x @ W and the device only does:
    sqd[i,j] = ri[i] + cj[j] - 2*gram[i,j]     (ri, cj host-precomputed)
    G0 = sqrt(sqd + CLAMP)                      (CLAMP covers tf32 noise on diag)
    rowmax[i] = max_j G0[i,j]
    out[i,:]  = w2 - (G0 @ xw)[i,:]/rowmax[i],  w2 = colsum_x @ W (host)

On device the distance strip is computed TRANSPOSED (sqd^T[j,i]) so the
G0 tiles come out with j (the contraction dim of Y = G0 @ xw) on
partitions -- no transposes of G0 needed. cj[j] rides the ACT sqrt bias
(per-partition); ri[i] varies along the free dim so it is added by DVE
from a host-replicated [128, R] tile (avoids a 512-cycle aug matmul per
j-tile). Each core sees its own np.roll'ed copy of the inputs so the
"local rows" are always rows [0,1024): one uniform SPMD program.

All matmuls use float32r (TF32 mode: 1 cycle/row at free dim >= 512).
"""

import numpy as np
from contextlib import ExitStack

import concourse.bass as bass
from concourse import bacc
import concourse.tile as tile
from concourse import mybir
from concourse.bass_utils import run_bass_kernel_spmd
from concourse.masks import make_identity

N, D, NOUT = 8192, 512, 512
M = 8                 # cores
R = N // M            # 1024 local rows per core
EPS = 1e-6
CLAMP = 0.3           # covers tf32 rounding noise on the diagonal; ~1e-4 rel effect off-diag
F32 = mybir.dt.float32
F32R = mybir.dt.float32r

KT = D // 128         # 4 contraction sub-tiles
NJT = N // 128        # 64 j tiles
IB = 512              # i block (free dim of the gram matmuls)
NIB = R // IB         # 2
NSUB = IB // 128      # 4 sub-tiles of 128 rows per i block

CH = 512              # S^T DMA chunk width (columns); chunk c covers j_tiles 4c..4c+3
NCH = N // CH


def build_kernel(ctx, tc, out_d, xw_d, s_d, cj_d, ri_d, w2_d):
    nc = tc.nc

    singles = ctx.enter_context(tc.tile_pool(name="singles", bufs=1))
    xt_pool = ctx.enter_context(tc.tile_pool(name="xt", bufs=4))
    g0_pool = ctx.enter_context(tc.tile_pool(name="g0", bufs=4))
    t_pool = ctx.enter_context(tc.tile_pool(name="t", bufs=3))
    ysc_pool = ctx.enter_context(tc.tile_pool(name="ysc", bufs=4))
    osb_pool = ctx.enter_context(tc.tile_pool(name="osb", bufs=4))
    sm_pool = ctx.enter_context(tc.tile_pool(name="sm", bufs=4))
    macc_pool = ctx.enter_context(tc.tile_pool(name="macc", bufs=2))
    ps_tr = ctx.enter_context(tc.tile_pool(name="ps_tr", bufs=1, space="PSUM"))
    ps_g = ctx.enter_context(tc.tile_pool(name="ps_g", bufs=3, space="PSUM"))
    ps_y = ctx.enter_context(tc.tile_pool(name="ps_y", bufs=1, space="PSUM"))

    # --- persistent SBUF tensors ---
    st = singles.tile([128, KT * N], F32R)            # S^T: [k*N + j] layout
    cj_sb = singles.tile([128, NJT], F32)             # cj[t*128+p] at [p, t]
    ri_sb = singles.tile([128, R], F32)               # -ri/2, replicated rows
    w2_sb = singles.tile([128, NOUT], F32)            # w2 replicated rows
    ident = singles.tile([128, 128], F32)

    make_identity(nc, ident[:])

    def load_st_chunk(c):
        for k in range(KT):
            nc.sync.dma_start(
                out=st[:, k * N + c * CH: k * N + (c + 1) * CH],
                in_=s_d[bass.ts(k, 128), c * CH:(c + 1) * CH].bitcast(F32R),
            )

    load_st_chunk(0)
    nc.sync.dma_start(out=cj_sb[:], in_=cj_d)
    nc.sync.dma_start(out=ri_sb[:], in_=ri_d)
    nc.sync.dma_start(out=w2_sb[:], in_=w2_d)

    # --- main: per i-block: gram strip -> G0 -> Y accum -> normalize ---
    for ib in range(NIB):
        icol0 = ib * IB  # local column offset into S^T / ri
        psy = [ps_y.tile([128, NOUT], F32, tag=f"y{s}", name=f"psy{s}")
               for s in range(NSUB)]
        macc = macc_pool.tile([128, IB], F32, tag="macc")

        for jt in range(NJT):
            xt = xt_pool.tile([128, NOUT], F32R, tag="xt")
            nc.sync.dma_start(out=xt[:], in_=xw_d[bass.ts(jt, 128), :].bitcast(F32R))

            if ib == 0:
                if jt == 0:
                    load_st_chunk(1)
                    load_st_chunk(2)
                elif jt % 4 == 0 and jt // 4 + 2 < NCH:
                    load_st_chunk(jt // 4 + 2)

            psg = ps_g.tile([128, IB], F32, tag="g")
            for k in range(KT):
                nc.tensor.matmul(
                    psg[:],
                    st[:, k * N + jt * 128: k * N + jt * 128 + 128],
                    st[:, k * N + icol0: k * N + icol0 + IB],
                    start=(k == 0),
                    stop=(k == KT - 1),
                )

            # t = psg + (-ri/2)  (free-dim-varying term, DVE broadcast-free add)
            t = t_pool.tile([128, IB], F32, tag="t")
            nc.vector.tensor_add(t[:], psg[:], ri_sb[:, icol0:icol0 + IB])

            # G0^T tile = sqrt(-2*t + cj[j])   (cj includes +CLAMP)
            g0 = g0_pool.tile([128, IB], F32R, tag="g0")
            nc.scalar.activation(
                out=g0[:], in_=t[:],
                func=mybir.ActivationFunctionType.Sqrt,
                bias=cj_sb[:, jt:jt + 1], scale=-2.0,
            )

            if jt == 0:
                nc.vector.tensor_copy(out=macc[:], in_=g0[:].bitcast(F32))
            else:
                nc.vector.tensor_max(macc[:], macc[:], g0[:].bitcast(F32))

            # software pipeline: issue Y matmuls one step behind the gram so
            # the PE fills the DVE/ACT latency with the next gram
            if jt > 0:
                pg0, pxt = prev
                for s in range(NSUB):
                    nc.tensor.matmul(
                        psy[s][:], pg0[:, bass.ts(s, 128)], pxt[:],
                        start=(jt == 1), stop=False,
                    )
            prev = (g0, xt)

        pg0, pxt = prev
        for s in range(NSUB):
            nc.tensor.matmul(
                psy[s][:], pg0[:, bass.ts(s, 128)], pxt[:],
                start=False, stop=True,
            )

        # tail: rowmax -> -1/rowmax -> ysc = -Y/rowmax -> out = w2 + ysc
        for s in range(NSUB):
            pst = ps_tr.tile([128, 128], F32, tag="tr")
            nc.tensor.transpose(pst[:], macc[:, bass.ts(s, 128)], ident[:])
            rm = sm_pool.tile([128, 1], F32, tag="rm")
            nc.vector.tensor_reduce(
                out=rm[:], in_=pst[:], axis=mybir.AxisListType.X,
                op=mybir.AluOpType.max,
            )
            nrm = sm_pool.tile([128, 1], F32, tag="nrm")
            nc.vector.tensor_scalar_mul(nrm[:], rm[:], -1.0)
            ninv = sm_pool.tile([128, 1], F32, tag="ninv")
            nc.vector.reciprocal(ninv[:], nrm[:])  # -1/rowmax

            ysc = ysc_pool.tile([128, NOUT], F32, tag="ysc", name=f"ysc{s}")
            nc.scalar.activation(
                out=ysc[:], in_=psy[s][:],
                func=mybir.ActivationFunctionType.Copy, scale=ninv[:],
            )
            osb = osb_pool.tile([128, NOUT], F32, tag="osb")
            nc.vector.tensor_add(osb[:], ysc[:], w2_sb[:])
            nc.sync.dma_start(out=out_d[bass.ts(ib * NSUB + s, 128), :], in_=osb[:])


_NC_CACHE = {}


def _build_nc():
    if "nc" in _NC_CACHE:
        return _NC_CACHE["nc"]
    nc = bacc.Bacc("TRN2", target_bir_lowering=False, debug=False, num_devices=M)
    xw_d = nc.dram_tensor("xw", [N, NOUT], F32, kind="ExternalInput").ap()
    s_d = nc.dram_tensor("simT", [D, N], F32, kind="ExternalInput").ap()
    cj_d = nc.dram_tensor("cj", [128, NJT], F32, kind="ExternalInput").ap()
    ri_d = nc.dram_tensor("rirep", [128, R], F32, kind="ExternalInput").ap()
    w2_d = nc.dram_tensor("w2rep", [128, NOUT], F32, kind="ExternalInput").ap()
    out_d = nc.dram_tensor("out", [R, NOUT], F32, kind="ExternalOutput").ap()
    with tile.TileContext(nc) as tc, ExitStack() as ctx:
        build_kernel(ctx, tc, out_d, xw_d, s_d, cj_d, ri_d, w2_d)
    nc.compile()
    _NC_CACHE["nc"] = nc
    return nc


def make_in_maps(x, sim_feat, weight):
    x = np.ascontiguousarray(x, dtype=np.float32)
    sim = np.ascontiguousarray(sim_feat, dtype=np.float32)
    w = np.ascontiguousarray(weight, dtype=np.float32)

    sim64 = sim.astype(np.float64)
    sq = (sim64 * sim64).sum(1)
    ss = sim64.sum(1)
    cj_full = (sq - 2.0 * EPS * ss + CLAMP).astype(np.float32)         # [N]
    ri_full = sq + 2.0 * EPS * ss + D * EPS * EPS                      # [N] f64
    colsum = x.astype(np.float64).sum(0)
    w2 = (colsum @ w.astype(np.float64)).astype(np.float32)
    xw = np.ascontiguousarray(x @ w)                                   # [N, NOUT] f32 sgemm
    w2_rep = np.ascontiguousarray(np.broadcast_to(w2, (128, NOUT)))

    in_maps = []
    for c in range(M):
        shift = c * R
        sim_c = np.ascontiguousarray(np.roll(sim, -shift, axis=0).T)
        xw_c = np.roll(xw, -shift, axis=0)
        cj_c = np.ascontiguousarray(
            np.roll(cj_full, -shift).reshape(NJT, 128).T
        )                                                               # [128, NJT]
        ri_c = np.ascontiguousarray(np.broadcast_to(
            (-(ri_full[shift:shift + R]) / 2.0).astype(np.float32), (128, R)
        ))
        in_maps.append(
            {"xw": xw_c, "simT": sim_c, "cj": cj_c, "rirep": ri_c,
             "w2rep": w2_rep}
        )
    return in_maps


def kernel(x, sim_feat, weight, _trace=False, **kw):
    nc = _build_nc()
    in_maps = make_in_maps(x, sim_feat, weight)
    res = run_bass_kernel_spmd(nc, in_maps, list(range(M)), trace=_trace, **kw)
    out = np.concatenate([res.results[c]["out"] for c in range(M)], axis=0)
    if _trace:
        return out, res
    return out
